# revision 9
# baseline (speedup 1.0000x reference)
"""LocalAutoCorr2D Trainium2 kernel.

out[b,c,i,j,dy,dx] = sum_{y,x valid} x[b,c,4i+y,4j+x] * x[b,c,4i+y+sy,4j+x+sx]
with (sy,sx) = (dy-4, dx-4), windows 8x8 at stride 4 on a 96x96 image,
zero-padded at window boundaries.

Strategy (per core, batch-sharded over 8 cores):
  - out[s] == out[-s] (autocorr symmetry) -> only 40 canonical shift classes.
  - x is host-prepped into PHASE-MAJOR fp16 layouts with the image split
    as u = 4g + r (r the phase, innermost axis c), so every matmul rhs
    view (fixed r, a 23-g window, all c) is one FLAT contiguous slice:
    the PE streams at full rate (a strided/multi-dim rhs runs ~2x slower).
    The 5 shift copies along the partition axis are host-stacked in the
    free dim, so DVE products never need cross-partition operands.
  - Per shift, the box-sum runs in the orientation that contracts the
    LONGER box extent through the 0/1 A-matrix matmul (partition axis)
    and accumulates the shorter extent via PSUM across passes:
    passes = min(8-|sy|, 8-|sx|). Two host layouts: xa ([h, ...] stack
    over sy) for horizontal-pass shifts, xw ([w, ...] stack over sx) for
    vertical-pass shifts. The A-matrix pattern is identical either way.
  - Products on the Vector engine (fp16 2x mode, flat contiguous views,
    all offsets multiples of C=64 so alignment is automatic); the (0,0)
    square runs on the Act engine. Warmup matmuls during the input DMA
    keep the PE p-state ramped. Scalar evacuates PSUM -> SBUF (fp16);
    GpSimd queues the output DMAs.
"""

import functools
import os
import sys

import numpy as np

sys.path.insert(0, "/opt/trn_rl_repo")

import concourse.bass as bass  # noqa: E402
import concourse.bacc as bacc  # noqa: E402
import concourse.mybir as mybir  # noqa: E402
from concourse import bass_utils  # noqa: E402
from concourse.tile import TileContext  # noqa: E402

B, C, H, W = 8, 64, 96, 96
KH = KW = 8
SH = SW = 4
NH = NW = 23
NCORES = 8

JP = 24           # g positions per r-block (u = 4g + r)
BLK = C * JP      # 1536 elements per r-block
FLAT = 4 * BLK    # 6144
NV = 5            # partition-shift copies v=0..4 stacked in the free dim
NVW = 4           # xw only needs v=0..3 (vertical-pass shifts have |sx|<=3)
BASE = 64         # leading pad elements (AP validity for negative offsets)
TAIL = 128
XCOLS = BASE + NV * FLAT + TAIL
XCOLSW = BASE + NVW * FLAT + TAIL
N_CHUNKS = [(0, 512), (512, 1024), (1024, 1472)]  # flat cols per PSUM bank
N_WARM = 48       # PE warmup matmuls issued under the input DMA

fp32 = mybir.dt.float32
fp16 = mybir.dt.float16


def _canonical_cells():
    """Map canonical shift (sy>=0, sx) -> list of output cells (dy,dx)."""
    cells = {}
    for dy in range(8):
        for dx in range(8):
            sy, sx = dy - 4, dx - 4
            key = (sy, sx) if (sy > 0 or (sy == 0 and sx >= 0)) else (-sy, -sx)
            cells.setdefault(key, []).append((dy, dx))
    assert len(cells) == 40
    return cells


def _is_w(key):
    """Vertical-pass (w-contracting) orientation when the x-extent of the
    box is shorter than the y-extent: passes = min of the two."""
    sy, sx = key
    return sy > abs(sx)


def _w_rep(key):
    """Representative (syw, sxw) with sxw >= 0 for the xw stack."""
    sy, sx = key
    return (sy, sx) if sx >= 0 else (-sy, -sx)


def _amat_np():
    """Box-sum matrices, stacked: A[u, p*23+g] = 1 if 0 <= u-4g < 8-p.
    Used as the vertical matrix (u=h, p=sy) and, identically, as the
    horizontal matrix (u=w, p=sxw)."""
    a = np.zeros((H, 5 * NH), np.float16)
    for p in range(5):
        for g in range(NH):
            a[4 * g : 4 * g + 8 - p, p * NH + g] = 1.0
    return a


def _stack(img, nv):
    """[U, V, C] fp32 (partition axis U first) -> phase-major fp16 stack
    [U, BASE + nv*FLAT + TAIL]: col v*FLAT + (r, g, c) = img[u+v, 4g+r, c]."""
    U = img.shape[0]
    pm = img.reshape(U, JP, 4, C).transpose(0, 2, 1, 3)  # [u, r, g, c]
    flat = np.ascontiguousarray(pm.reshape(U, FLAT)).astype(np.float16)
    out = np.zeros((U, BASE + nv * FLAT + TAIL), np.float16)
    for v in range(nv):
        out[0 : U - v, BASE + v * FLAT : BASE + (v + 1) * FLAT] = flat[v:U]
    return out


def _prep_x(xb):
    """[C,H,W] fp32 -> (xa [H, XCOLS], xw [W, XCOLSW]) fp16 stacks."""
    xa = _stack(xb.transpose(1, 2, 0), NV)        # [h, w, c] stack over sy
    xw = _stack(xb.transpose(2, 1, 0), NVW)       # [w, h, c] stack over sx
    return xa, xw


def _order(cells):
    """All H-orientation shifts first (they only need xa, whose chunks
    land first), ordered sy-major so early shifts need only low-v blocks;
    then the W-orientation shifts once xw has landed."""
    ks = sorted(cells.keys(), key=lambda s: (_is_w(s), s[0], abs(s[1])))
    return ks


def build_nc():
    nc = bacc.Bacc()
    xa_dram = nc.dram_tensor("xa", [H, XCOLS], fp16, kind="ExternalInput")
    xw_dram = nc.dram_tensor("xw", [W, XCOLSW], fp16, kind="ExternalInput")
    amat_dram = nc.dram_tensor("amat", [H, 5 * NH], fp16, kind="ExternalInput")
    out_dram = nc.dram_tensor("out", [8, 8, NH, NW * C], fp16,
                              kind="ExternalOutput")

    cells = _canonical_cells()
    order = _order(cells)

    with TileContext(nc) as tc:
        with (
            tc.tile_pool(name="const", bufs=1) as cpool,
            tc.tile_pool(name="q", bufs=5) as qpool,
            tc.tile_pool(name="o", bufs=3) as opool,
            tc.tile_pool(name="ps", bufs=2, space="PSUM") as ppool,
            tc.tile_pool(name="pw", bufs=1, space="PSUM") as wpool,
        ):
            amat_t = cpool.tile([H, 5 * NH], fp16)
            nc.gpsimd.dma_start(amat_t, amat_dram[:, :])
            xa_t = cpool.tile([H, XCOLS], fp16)
            xw_t = cpool.tile([W, XCOLSW], fp16)
            # PE warmup: keep the p-state ramped while inputs stream in
            # (memset on the otherwise-idle DVE so warmup starts at once)
            wt = cpool.tile([H, 512], fp16)
            nc.vector.memset(wt, 0.0)
            warm_pt = wpool.tile([NH, 512], fp32)
            for _ in range(N_WARM):
                nc.tensor.matmul(warm_pt, wt[:, 0:NH], wt,
                                 start=True, stop=True)

            # lowest v first so early consumers unblock first; all xa
            # (H-shifts run first), then xw; xa v=0 in halves so the
            # (0,0) square can start on the first half
            bounds_a = [0, BASE + FLAT // 2] + \
                [BASE + v * FLAT for v in range(1, NV)] + [XCOLS]
            bounds_w = [0] + [BASE + v * FLAT for v in range(1, NVW)] + [XCOLSW]
            dmas = [(xa_t, xa_dram, lo, hi)
                    for lo, hi in zip(bounds_a[:-1], bounds_a[1:])]
            dmas += [(xw_t, xw_dram, lo, hi)
                     for lo, hi in zip(bounds_w[:-1], bounds_w[1:])]
            for t, dram, lo, hi in dmas:
                nc.gpsimd.dma_start(t[:, lo:hi], dram[:, lo:hi])

            for key in order:
                sy, sx = key
                if _is_w(key):
                    syw, sxw = _w_rep(key)
                    p_shift, f_shift = sxw, syw
                    stack_t = xw_t
                    passes = list(range(max(0, -syw), 8 - max(0, syw)))
                else:
                    p_shift, f_shift = sy, sx
                    stack_t = xa_t
                    passes = list(range(max(0, -sx), 8 - max(0, sx)))
                s = f_shift % 4          # python %: s in [0,4) for negatives
                a = (f_shift - s) // 4
                pv = H - p_shift
                q = qpool.tile([H, FLAT], fp16, tag="q")

                def mul(flo, fhi, delta):
                    nc.vector.tensor_mul(
                        q[0:pv, flo:fhi],
                        stack_t[0:pv, BASE + flo : BASE + fhi],
                        stack_t[0:pv, BASE + delta + flo : BASE + delta + fhi],
                    )

                if key == (0, 0):
                    # x^2 on the Act engine: frees the DVE and starts as
                    # soon as each half of the v=0 DMA chunk lands
                    for lo, hi in [(0, FLAT // 2), (FLAT // 2, FLAT)]:
                        nc.scalar.activation(
                            q[:, lo:hi], xa_t[:, BASE + lo : BASE + hi],
                            mybir.ActivationFunctionType.Square,
                        )
                else:
                    lenA = (4 - s) * BLK
                    mul(0, lenA, p_shift * FLAT + s * BLK + C * a)
                    if s:
                        mul(lenA, FLAT,
                            p_shift * FLAT + (s - 4) * BLK + C * (a + 1))

                a_k = amat_t[0:pv, p_shift * NH : (p_shift + 1) * NH]
                o_t = opool.tile([NH, NW * C], fp16, tag="o")
                for ci, (n0, n1) in enumerate(N_CHUNKS):
                    pt = ppool.tile([NH, n1 - n0], fp32, tag=f"ps{ci}")
                    for pi, y in enumerate(passes):
                        base = (y % 4) * BLK + C * (y // 4)
                        rhs = q[0:pv, base + n0 : base + n1]
                        nc.tensor.matmul(
                            pt, a_k, rhs,
                            start=(pi == 0), stop=(pi == len(passes) - 1),
                        )
                    nc.scalar.copy(o_t[:, n0:n1], pt)
                for (dy, dx) in cells[key]:
                    nc.gpsimd.dma_start(out_dram[dy, dx], o_t)

    if not nc.is_finalized():
        nc.finalize()
    return nc


@functools.lru_cache(maxsize=1)
def _get_nc():
    return build_nc()


def _in_maps(x):
    amat = _amat_np()
    maps = []
    for b in range(NCORES):
        xa, xw = _prep_x(x[b])
        maps.append({"xa": xa, "xw": xw, "amat": amat})
    return maps


def _w_cells():
    cells = _canonical_cells()
    out = set()
    for key, cs in cells.items():
        if _is_w(key):
            out.update(cs)
    return out


def kernel(**inputs) -> np.ndarray:
    x = np.asarray(inputs["x"], dtype=np.float32)
    assert x.shape == (B, C, H, W)
    nc = _get_nc()
    in_maps = _in_maps(x)
    res = bass_utils.run_bass_kernel_spmd(
        nc, in_maps, core_ids=list(range(NCORES)),
        trace=bool(int(os.environ.get("KERNEL_TRACE", "0"))),
    )
    outs = np.stack([r["out"] for r in res.results])  # [B, dy, dx, ?, ?]
    outs = outs.reshape(B, 8, 8, NH, NW, C).astype(np.float32)
    # w-oriented cells come out [j, i, c]: swap back to [i, j, c]
    wc = _w_cells()
    full = outs.copy()
    for (dy, dx) in wc:
        full[:, dy, dx] = outs[:, dy, dx].transpose(0, 2, 1, 3)
    # [B, dy, dx, i, j, c] -> [B, c, i, j, dy, dx]
    full = full.transpose(0, 5, 3, 4, 1, 2)
    return np.ascontiguousarray(full).astype(np.float32)


if __name__ == "__main__":
    rng = np.random.default_rng(0)
    x = rng.standard_normal((B, C, H, W), dtype=np.float32)
    y = kernel(x=x)
    print("out", y.shape, y.dtype, float(np.abs(y).max()))


# revision 10
# speedup vs baseline: 1.0089x; 1.0089x over previous
"""LocalAutoCorr2D Trainium2 kernel.

out[b,c,i,j,dy,dx] = sum_{y,x valid} x[b,c,4i+y,4j+x] * x[b,c,4i+y+sy,4j+x+sx]
with (sy,sx) = (dy-4, dx-4), windows 8x8 at stride 4 on a 96x96 image,
zero-padded at window boundaries.

Strategy (per core, batch-sharded over 8 cores):
  - out[s] == out[-s] (autocorr symmetry) -> only 40 canonical shift classes.
  - x is host-prepped into PHASE-MAJOR fp16 layouts with the image split
    as u = 4g + r (r the phase, innermost axis c), so every matmul rhs
    view (fixed r, a 23-g window, all c) is one FLAT contiguous slice:
    the PE streams at full rate (a strided/multi-dim rhs runs ~2x slower).
    The 5 shift copies along the partition axis are host-stacked in the
    free dim, so DVE products never need cross-partition operands.
  - Per shift, the box-sum runs in the orientation that contracts the
    LONGER box extent through the 0/1 A-matrix matmul (partition axis)
    and accumulates the shorter extent via PSUM across passes:
    passes = min(8-|sy|, 8-|sx|). Two host layouts: xa ([h, ...] stack
    over sy) for horizontal-pass shifts, xw ([w, ...] stack over sx) for
    vertical-pass shifts. The A-matrix pattern is identical either way.
  - Products on the Vector engine (fp16 2x mode, flat contiguous views,
    all offsets multiples of C=64 so alignment is automatic); the (0,0)
    square runs on the Act engine. Warmup matmuls during the input DMA
    keep the PE p-state ramped. Scalar evacuates PSUM -> SBUF (fp16);
    GpSimd queues the output DMAs.
"""

import functools
import os
import sys

import numpy as np

sys.path.insert(0, "/opt/trn_rl_repo")

import concourse.bass as bass  # noqa: E402
import concourse.bacc as bacc  # noqa: E402
import concourse.mybir as mybir  # noqa: E402
from concourse import bass_utils  # noqa: E402
from concourse.tile import TileContext  # noqa: E402

B, C, H, W = 8, 64, 96, 96
KH = KW = 8
SH = SW = 4
NH = NW = 23
NCORES = 8

JP = 24           # g positions per r-block (u = 4g + r)
BLK = C * JP      # 1536 elements per r-block
FLAT = 4 * BLK    # 6144
NV = 5            # partition-shift copies v=0..4 stacked in the free dim
NVW = 4           # xw only needs v=0..3 (vertical-pass shifts have |sx|<=3)
BASE = 64         # leading pad elements (AP validity for negative offsets)
TAIL = 128
XCOLS = BASE + NV * FLAT + TAIL
XCOLSW = BASE + NVW * FLAT + TAIL
N_CHUNKS = [(0, 512), (512, 1024), (1024, 1472)]  # flat cols per PSUM bank
N_WARM = 48       # PE warmup matmuls issued under the input DMA

fp32 = mybir.dt.float32
fp16 = mybir.dt.float16


def _canonical_cells():
    """Map canonical shift (sy>=0, sx) -> list of output cells (dy,dx)."""
    cells = {}
    for dy in range(8):
        for dx in range(8):
            sy, sx = dy - 4, dx - 4
            key = (sy, sx) if (sy > 0 or (sy == 0 and sx >= 0)) else (-sy, -sx)
            cells.setdefault(key, []).append((dy, dx))
    assert len(cells) == 40
    return cells


def _is_w(key):
    """Vertical-pass (w-contracting) orientation when the x-extent of the
    box is shorter than the y-extent: passes = min of the two."""
    sy, sx = key
    return sy > abs(sx)


def _w_rep(key):
    """Representative (syw, sxw) with sxw >= 0 for the xw stack."""
    sy, sx = key
    return (sy, sx) if sx >= 0 else (-sy, -sx)


def _amat_np():
    """Box-sum matrices, stacked: A[u, p*23+g] = 1 if 0 <= u-4g < 8-p.
    Used as the vertical matrix (u=h, p=sy) and, identically, as the
    horizontal matrix (u=w, p=sxw)."""
    a = np.zeros((H, 5 * NH), np.float16)
    for p in range(5):
        for g in range(NH):
            a[4 * g : 4 * g + 8 - p, p * NH + g] = 1.0
    return a


def _stack(img, nv):
    """[U, V, C] fp32 (partition axis U first) -> phase-major fp16 stack
    [U, BASE + nv*FLAT + TAIL]: col v*FLAT + (r, g, c) = img[u+v, 4g+r, c]."""
    U = img.shape[0]
    pm = img.reshape(U, JP, 4, C).transpose(0, 2, 1, 3)  # [u, r, g, c]
    flat = np.ascontiguousarray(pm.reshape(U, FLAT)).astype(np.float16)
    out = np.zeros((U, BASE + nv * FLAT + TAIL), np.float16)
    for v in range(nv):
        out[0 : U - v, BASE + v * FLAT : BASE + (v + 1) * FLAT] = flat[v:U]
    return out


def _prep_x(xb):
    """[C,H,W] fp32 -> (xa [H, XCOLS], xw [W, XCOLSW]) fp16 stacks."""
    xa = _stack(xb.transpose(1, 2, 0), NV)        # [h, w, c] stack over sy
    xw = _stack(xb.transpose(2, 1, 0), NVW)       # [w, h, c] stack over sx
    return xa, xw


def _order(cells):
    """All H-orientation shifts first (they only need xa, whose chunks
    land first), ordered sy-major so early shifts need only low-v blocks;
    then the W-orientation shifts once xw has landed."""
    ks = sorted(cells.keys(), key=lambda s: (_is_w(s), s[0], abs(s[1])))
    return ks


def build_nc():
    nc = bacc.Bacc()
    xa_dram = nc.dram_tensor("xa", [H, XCOLS], fp16, kind="ExternalInput")
    xw_dram = nc.dram_tensor("xw", [W, XCOLSW], fp16, kind="ExternalInput")
    amat_dram = nc.dram_tensor("amat", [H, 5 * NH], fp16, kind="ExternalInput")
    out_dram = nc.dram_tensor("out", [8, 8, NH, NW * C], fp16,
                              kind="ExternalOutput")

    cells = _canonical_cells()
    order = _order(cells)

    with TileContext(nc) as tc:
        with (
            tc.tile_pool(name="const", bufs=1) as cpool,
            tc.tile_pool(name="q", bufs=4) as qpool,
            tc.tile_pool(name="o", bufs=3) as opool,
            tc.tile_pool(name="ps", bufs=2, space="PSUM") as ppool,
            tc.tile_pool(name="pw", bufs=1, space="PSUM") as wpool,
        ):
            amat_t = cpool.tile([H, 5 * NH], fp16)
            nc.gpsimd.dma_start(amat_t, amat_dram[:, :])
            xa_t = cpool.tile([H, XCOLS], fp16)
            xw_t = cpool.tile([W, XCOLSW], fp16)
            # PE warmup: keep the p-state ramped while inputs stream in
            # (memset on the otherwise-idle DVE so warmup starts at once)
            wt = cpool.tile([H, 512], fp16)
            nc.vector.memset(wt, 0.0)
            warm_pt = wpool.tile([NH, 512], fp32)
            for _ in range(N_WARM):
                nc.tensor.matmul(warm_pt, wt[:, 0:NH], wt,
                                 start=True, stop=True)

            # lowest v first so early consumers unblock first; all xa
            # (H-shifts run first), then xw; xa v=0 in halves so the
            # (0,0) square can start on the first half
            bounds_a = [0, BASE + FLAT // 2] + \
                [BASE + v * FLAT for v in range(1, NV)] + [XCOLS]
            bounds_w = [0] + [BASE + v * FLAT for v in range(1, NVW)] + [XCOLSW]
            dmas = [(xa_t, xa_dram, lo, hi)
                    for lo, hi in zip(bounds_a[:-1], bounds_a[1:])]
            dmas += [(xw_t, xw_dram, lo, hi)
                     for lo, hi in zip(bounds_w[:-1], bounds_w[1:])]
            for t, dram, lo, hi in dmas:
                nc.gpsimd.dma_start(t[:, lo:hi], dram[:, lo:hi])

            for key in order:
                sy, sx = key
                if _is_w(key):
                    syw, sxw = _w_rep(key)
                    p_shift, f_shift = sxw, syw
                    stack_t = xw_t
                    passes = list(range(max(0, -syw), 8 - max(0, syw)))
                else:
                    p_shift, f_shift = sy, sx
                    stack_t = xa_t
                    passes = list(range(max(0, -sx), 8 - max(0, sx)))
                s = f_shift % 4          # python %: s in [0,4) for negatives
                a = (f_shift - s) // 4
                pv = H - p_shift
                q = qpool.tile([H, FLAT], fp16, tag="q")

                def mul(flo, fhi, delta):
                    nc.vector.tensor_mul(
                        q[0:pv, flo:fhi],
                        stack_t[0:pv, BASE + flo : BASE + fhi],
                        stack_t[0:pv, BASE + delta + flo : BASE + delta + fhi],
                    )

                if key == (0, 0):
                    # x^2 on the Act engine: frees the DVE and starts as
                    # soon as each half of the v=0 DMA chunk lands
                    for lo, hi in [(0, FLAT // 2), (FLAT // 2, FLAT)]:
                        nc.scalar.activation(
                            q[:, lo:hi], xa_t[:, BASE + lo : BASE + hi],
                            mybir.ActivationFunctionType.Square,
                        )
                else:
                    lenA = (4 - s) * BLK
                    mul(0, lenA, p_shift * FLAT + s * BLK + C * a)
                    if s:
                        mul(lenA, FLAT,
                            p_shift * FLAT + (s - 4) * BLK + C * (a + 1))

                a_k = amat_t[0:pv, p_shift * NH : (p_shift + 1) * NH]
                o_t = opool.tile([NH, NW * C], fp16, tag="o")
                for ci, (n0, n1) in enumerate(N_CHUNKS):
                    pt = ppool.tile([NH, n1 - n0], fp32, tag=f"ps{ci}")
                    for pi, y in enumerate(passes):
                        base = (y % 4) * BLK + C * (y // 4)
                        rhs = q[0:pv, base + n0 : base + n1]
                        nc.tensor.matmul(
                            pt, a_k, rhs,
                            start=(pi == 0), stop=(pi == len(passes) - 1),
                        )
                    nc.scalar.copy(o_t[:, n0:n1], pt)
                for (dy, dx) in cells[key]:
                    nc.gpsimd.dma_start(out_dram[dy, dx], o_t)

    if not nc.is_finalized():
        nc.finalize()
    return nc


@functools.lru_cache(maxsize=1)
def _get_nc():
    return build_nc()


def _in_maps(x):
    amat = _amat_np()
    maps = []
    for b in range(NCORES):
        xa, xw = _prep_x(x[b])
        maps.append({"xa": xa, "xw": xw, "amat": amat})
    return maps


def _w_cells():
    cells = _canonical_cells()
    out = set()
    for key, cs in cells.items():
        if _is_w(key):
            out.update(cs)
    return out


def kernel(**inputs) -> np.ndarray:
    x = np.asarray(inputs["x"], dtype=np.float32)
    assert x.shape == (B, C, H, W)
    nc = _get_nc()
    in_maps = _in_maps(x)
    res = bass_utils.run_bass_kernel_spmd(
        nc, in_maps, core_ids=list(range(NCORES)),
        trace=bool(int(os.environ.get("KERNEL_TRACE", "0"))),
    )
    outs = np.stack([r["out"] for r in res.results])  # [B, dy, dx, ?, ?]
    outs = outs.reshape(B, 8, 8, NH, NW, C).astype(np.float32)
    # w-oriented cells come out [j, i, c]: swap back to [i, j, c]
    wc = _w_cells()
    full = outs.copy()
    for (dy, dx) in wc:
        full[:, dy, dx] = outs[:, dy, dx].transpose(0, 2, 1, 3)
    # [B, dy, dx, i, j, c] -> [B, c, i, j, dy, dx]
    full = full.transpose(0, 5, 3, 4, 1, 2)
    return np.ascontiguousarray(full).astype(np.float32)


if __name__ == "__main__":
    rng = np.random.default_rng(0)
    x = rng.standard_normal((B, C, H, W), dtype=np.float32)
    y = kernel(x=x)
    print("out", y.shape, y.dtype, float(np.abs(y).max()))


# revision 11
# speedup vs baseline: 1.3000x; 1.2885x over previous
"""LocalAutoCorr2D Trainium2 kernel.

out[b,c,i,j,dy,dx] = sum_{y,x valid} x[b,c,4i+y,4j+x] * x[b,c,4i+y+sy,4j+x+sx]
with (sy,sx) = (dy-4, dx-4), windows 8x8 at stride 4 on a 96x96 image,
zero-padded at window boundaries.

Strategy (per core, batch-sharded over 8 cores):
  - out[s] == out[-s] (autocorr symmetry) -> only 40 canonical shift classes.
  - x is host-prepped into PHASE-MAJOR fp16 layouts with the image split
    as u = 4g + r (r the phase, innermost axis c), so every matmul rhs
    view (fixed r, a 23-g window, all c) is one FLAT contiguous slice:
    the PE streams at full rate (a strided/multi-dim rhs runs ~2x slower).
    The 5 shift copies along the partition axis are host-stacked in the
    free dim, so DVE products never need cross-partition operands.
  - Per shift, the box-sum runs in the orientation that contracts the
    LONGER box extent through the 0/1 A-matrix matmul (partition axis)
    and accumulates the shorter extent via PSUM across passes:
    passes = min(8-|sy|, 8-|sx|). Two host layouts: xa ([h, ...] stack
    over sy) for horizontal-pass shifts, xw ([w, ...] stack over sx) for
    vertical-pass shifts. The A-matrix pattern is identical either way.
  - Products on the Vector engine (fp16 2x mode, flat contiguous views,
    all offsets multiples of C=64 so alignment is automatic); the (0,0)
    square runs on the Act engine. Warmup matmuls during the input DMA
    keep the PE p-state ramped. Scalar evacuates PSUM -> SBUF (fp16);
    GpSimd queues the output DMAs.
"""

import functools
import os
import sys

import numpy as np

sys.path.insert(0, "/opt/trn_rl_repo")

import concourse.bass as bass  # noqa: E402
import concourse.bacc as bacc  # noqa: E402
import concourse.mybir as mybir  # noqa: E402
from concourse import bass_utils  # noqa: E402
from concourse.tile import TileContext  # noqa: E402

B, C, H, W = 8, 64, 96, 96
KH = KW = 8
SH = SW = 4
NH = NW = 23
NCORES = 8

JP = 24           # g positions per r-block (u = 4g + r)
BLK = C * JP      # 1536 elements per r-block
FLAT = 4 * BLK    # 6144
NV = 5            # partition-shift copies v=0..4 stacked in the free dim
NVW = 4           # xw only needs v=0..3 (vertical-pass shifts have |sx|<=3)
BASE = 64         # leading pad elements (AP validity for negative offsets)
TAIL = 128
XCOLS = BASE + NV * FLAT + TAIL
XCOLSW = BASE + NVW * FLAT + TAIL
N_CHUNKS = [(0, 512), (512, 1024), (1024, 1472)]  # flat cols per PSUM bank
N_WARM = 48       # PE warmup matmuls issued under the input DMA

fp32 = mybir.dt.float32
fp16 = mybir.dt.float16


def _canonical_cells():
    """Map canonical shift (sy>=0, sx) -> list of output cells (dy,dx)."""
    cells = {}
    for dy in range(8):
        for dx in range(8):
            sy, sx = dy - 4, dx - 4
            key = (sy, sx) if (sy > 0 or (sy == 0 and sx >= 0)) else (-sy, -sx)
            cells.setdefault(key, []).append((dy, dx))
    assert len(cells) == 40
    return cells


def _is_w(key):
    """Vertical-pass (w-contracting) orientation when the x-extent of the
    box is shorter than the y-extent: passes = min of the two."""
    sy, sx = key
    return sy > abs(sx)


def _w_rep(key):
    """Representative (syw, sxw) with sxw >= 0 for the xw stack."""
    sy, sx = key
    return (sy, sx) if sx >= 0 else (-sy, -sx)


def _amat_np():
    """Box-sum matrices, stacked: A[u, p*23+g] = 1 if 0 <= u-4g < 8-p.
    Used as the vertical matrix (u=h, p=sy) and, identically, as the
    horizontal matrix (u=w, p=sxw)."""
    a = np.zeros((H, 5 * NH), np.float16)
    for p in range(5):
        for g in range(NH):
            a[4 * g : 4 * g + 8 - p, p * NH + g] = 1.0
    return a


def _stack(img, nv):
    """[U, V, C] fp32 (partition axis U first) -> phase-major fp16 stack
    [U, BASE + nv*FLAT + TAIL]: col v*FLAT + (r, g, c) = img[u+v, 4g+r, c]."""
    U = img.shape[0]
    pm = img.reshape(U, JP, 4, C).transpose(0, 2, 1, 3)  # [u, r, g, c]
    flat = np.ascontiguousarray(pm.reshape(U, FLAT)).astype(np.float16)
    out = np.zeros((U, BASE + nv * FLAT + TAIL), np.float16)
    for v in range(nv):
        out[0 : U - v, BASE + v * FLAT : BASE + (v + 1) * FLAT] = flat[v:U]
    return out


def _prep_x(xb):
    """[C,H,W] fp32 -> (xa [H, XCOLS], xw [W, XCOLSW]) fp16 stacks."""
    xa = _stack(xb.transpose(1, 2, 0), NV)        # [h, w, c] stack over sy
    xw = _stack(xb.transpose(2, 1, 0), NVW)       # [w, h, c] stack over sx
    return xa, xw


def _order(cells):
    """sy-major (low-v stack blocks land first), |sx| ascending; (4,0)
    moved to the very end so the PE drains on real work."""
    o = sorted(cells.keys(), key=lambda s: (s[0], abs(s[1])))
    o.remove((4, 0))
    o.append((4, 0))
    return o


def build_nc():
    nc = bacc.Bacc()
    xa_dram = nc.dram_tensor("xa", [H, XCOLS], fp16, kind="ExternalInput")
    xw_dram = nc.dram_tensor("xw", [W, XCOLSW], fp16, kind="ExternalInput")
    amat_dram = nc.dram_tensor("amat", [H, 5 * NH], fp16, kind="ExternalInput")
    out_dram = nc.dram_tensor("out", [8, 8, NH, NW * C], fp16,
                              kind="ExternalOutput")

    cells = _canonical_cells()
    order = _order(cells)

    with TileContext(nc) as tc:
        with (
            tc.tile_pool(name="const", bufs=1) as cpool,
            tc.tile_pool(name="q", bufs=4) as qpool,
            tc.tile_pool(name="o", bufs=3) as opool,
            tc.tile_pool(name="ps", bufs=2, space="PSUM") as ppool,
            tc.tile_pool(name="pw", bufs=1, space="PSUM") as wpool,
        ):
            amat_t = cpool.tile([H, 5 * NH], fp16)
            nc.gpsimd.dma_start(amat_t, amat_dram[:, :])
            xa_t = cpool.tile([H, XCOLS], fp16)
            xw_t = cpool.tile([W, XCOLSW], fp16)
            # lowest v first so early consumers unblock first; xw v=0
            # right after xa v=0 (the (1,0) W-shift is the 6th consumer);
            # xa v=0 in halves so the (0,0) square starts on the first
            ca = [(xa_t, xa_dram, lo, hi) for lo, hi in zip(
                [0, BASE + FLAT // 2] +
                [BASE + v * FLAT for v in range(1, NV)] + [XCOLS][:1],
                [BASE + FLAT // 2] +
                [BASE + v * FLAT for v in range(1, NV)] + [XCOLS])]
            cw = [(xw_t, xw_dram, lo, hi) for lo, hi in zip(
                [0] + [BASE + v * FLAT for v in range(1, NVW)],
                [BASE + v * FLAT for v in range(1, NVW)] + [XCOLSW])]
            dmas = [ca[0], ca[1], cw[0], ca[2], ca[3], cw[1], ca[4],
                    cw[2], ca[5], cw[3]]
            for t, dram, lo, hi in dmas:
                nc.gpsimd.dma_start(t[:, lo:hi], dram[:, lo:hi])

            # PE warmup: keep the p-state ramped while inputs stream in
            # (memset on the otherwise-idle DVE so warmup starts at once)
            wt = cpool.tile([H, 512], fp16)
            nc.vector.memset(wt, 0.0)
            warm_pt = wpool.tile([NH, 512], fp32)
            for _ in range(N_WARM):
                nc.tensor.matmul(warm_pt, wt[:, 0:NH], wt,
                                 start=True, stop=True)

            for key in order:
                sy, sx = key
                if _is_w(key):
                    syw, sxw = _w_rep(key)
                    p_shift, f_shift = sxw, syw
                    stack_t = xw_t
                    passes = list(range(max(0, -syw), 8 - max(0, syw)))
                else:
                    p_shift, f_shift = sy, sx
                    stack_t = xa_t
                    passes = list(range(max(0, -sx), 8 - max(0, sx)))
                s = f_shift % 4          # python %: s in [0,4) for negatives
                a = (f_shift - s) // 4
                pv = H - p_shift
                q = qpool.tile([H, FLAT], fp16, tag="q")

                def mul(flo, fhi, delta):
                    nc.vector.tensor_mul(
                        q[0:pv, flo:fhi],
                        stack_t[0:pv, BASE + flo : BASE + fhi],
                        stack_t[0:pv, BASE + delta + flo : BASE + delta + fhi],
                    )

                if key == (0, 0):
                    # x^2 on the Act engine: frees the DVE and starts as
                    # soon as each half of the v=0 DMA chunk lands
                    for lo, hi in [(0, FLAT // 2), (FLAT // 2, FLAT)]:
                        nc.scalar.activation(
                            q[:, lo:hi], xa_t[:, BASE + lo : BASE + hi],
                            mybir.ActivationFunctionType.Square,
                        )
                else:
                    lenA = (4 - s) * BLK
                    mul(0, lenA, p_shift * FLAT + s * BLK + C * a)
                    if s:
                        mul(lenA, FLAT,
                            p_shift * FLAT + (s - 4) * BLK + C * (a + 1))

                a_k = amat_t[0:pv, p_shift * NH : (p_shift + 1) * NH]
                o_t = opool.tile([NH, NW * C], fp16, tag="o")
                for ci, (n0, n1) in enumerate(N_CHUNKS):
                    pt = ppool.tile([NH, n1 - n0], fp32, tag=f"ps{ci}")
                    for pi, y in enumerate(passes):
                        base = (y % 4) * BLK + C * (y // 4)
                        rhs = q[0:pv, base + n0 : base + n1]
                        nc.tensor.matmul(
                            pt, a_k, rhs,
                            start=(pi == 0), stop=(pi == len(passes) - 1),
                        )
                    nc.scalar.copy(o_t[:, n0:n1], pt)
                for (dy, dx) in cells[key]:
                    nc.gpsimd.dma_start(out_dram[dy, dx], o_t)

    if not nc.is_finalized():
        nc.finalize()
    return nc


@functools.lru_cache(maxsize=1)
def _get_nc():
    return build_nc()


def _in_maps(x):
    amat = _amat_np()
    maps = []
    for b in range(NCORES):
        xa, xw = _prep_x(x[b])
        maps.append({"xa": xa, "xw": xw, "amat": amat})
    return maps


def _w_cells():
    cells = _canonical_cells()
    out = set()
    for key, cs in cells.items():
        if _is_w(key):
            out.update(cs)
    return out


def kernel(**inputs) -> np.ndarray:
    x = np.asarray(inputs["x"], dtype=np.float32)
    assert x.shape == (B, C, H, W)
    nc = _get_nc()
    in_maps = _in_maps(x)
    res = bass_utils.run_bass_kernel_spmd(
        nc, in_maps, core_ids=list(range(NCORES)),
        trace=bool(int(os.environ.get("KERNEL_TRACE", "0"))),
    )
    outs = np.stack([r["out"] for r in res.results])  # [B, dy, dx, ?, ?]
    outs = outs.reshape(B, 8, 8, NH, NW, C).astype(np.float32)
    # w-oriented cells come out [j, i, c]: swap back to [i, j, c]
    wc = _w_cells()
    full = outs.copy()
    for (dy, dx) in wc:
        full[:, dy, dx] = outs[:, dy, dx].transpose(0, 2, 1, 3)
    # [B, dy, dx, i, j, c] -> [B, c, i, j, dy, dx]
    full = full.transpose(0, 5, 3, 4, 1, 2)
    return np.ascontiguousarray(full).astype(np.float32)


if __name__ == "__main__":
    rng = np.random.default_rng(0)
    x = rng.standard_normal((B, C, H, W), dtype=np.float32)
    y = kernel(x=x)
    print("out", y.shape, y.dtype, float(np.abs(y).max()))
